# revision 5
# baseline (speedup 1.0000x reference)
"""Trainium2 Bass kernel for nn_BlockDiagonal.

Math: out = x @ tanh(W * mask).T, where mask is block-diagonal with 64
blocks of 64x64. tanh(0) = 0, so the effective weight is block-diagonal:
out[:, 64j:64j+64] = x[:, 64j:64j+64] @ tanh(Wb_j).T for each block j.

Strategy (8 NeuronCores, data parallel over rows):
- Each core owns 1024 rows of x / out.
- Host (numpy, data movement only): verify the mask is block-diagonal,
  gather the 64 diagonal 64x64 blocks of W, lay them out as 32 groups of
  two blocks packed into [128,128] block-diagonal tiles, TRANSPOSED, with
  zeros off-block -> wt [128, 4096]. tanh on device maps zeros to zeros.
- Device per core: tanh(wt) once; stream x in [128, 4096] chunks;
  PE-transpose each [128,128] x tile (fp32 transpose mode); fp32 matmuls
  lhsT = xT_tile [feat, rows], rhs = tanh-tile [feat_in, feat_out];
  copy PSUM->SBUF; DMA the [128, 4096] out chunk back.
"""

import os
import sys

import numpy as np

for _p in ("/opt/trn_rl_repo", "/root/.axon_site/_ro/trn_rl_repo"):
    if os.path.isdir(_p) and _p not in sys.path:
        sys.path.append(_p)

from contextlib import ExitStack

import concourse.bacc as bacc
from concourse import mybir
from concourse.bass_utils import run_bass_kernel_spmd
from concourse.tile import TileContext

ROWS = 8192
L = 4096
NB = 64           # number of diagonal blocks
B = 64            # block size
NCORES = 8
R = ROWS // NCORES  # rows per core = 1024
P = 128             # partition tile
NG = L // P         # 32 column groups (2 blocks each)
NCHUNK = R // P     # 8 row chunks per core

F32 = mybir.dt.float32

_NC_CACHE = {}


def build_nc():
    """Build the per-core Bass program (SPMD: same program, different data)."""
    nc = bacc.Bacc()
    x_in = nc.declare_dram_parameter("x", [R, L], F32, isOutput=False)
    wt_in = nc.declare_dram_parameter("wt", [P, L], F32, isOutput=False)
    id_in = nc.declare_dram_parameter("ident", [P, P], F32, isOutput=False)
    out_ext = nc.declare_dram_parameter("out", [R, L], F32, isOutput=True)

    with TileContext(nc) as tc, ExitStack() as ctx:
        singles = ctx.enter_context(tc.tile_pool(name="singles", bufs=1))
        xpool = ctx.enter_context(tc.tile_pool(name="xpool", bufs=4))
        xtpool = ctx.enter_context(tc.tile_pool(name="xtpool", bufs=6))
        opool = ctx.enter_context(tc.tile_pool(name="opool", bufs=3))
        pt = ctx.enter_context(tc.tile_pool(name="pt", bufs=2, space="PSUM"))
        pm = ctx.enter_context(tc.tile_pool(name="pm", bufs=2, space="PSUM"))

        ident = singles.tile([P, P], F32)
        nc.sync.dma_start(out=ident, in_=id_in[:])

        bt = singles.tile([P, L], F32)
        nc.sync.dma_start(out=bt, in_=wt_in[:])
        for q in range(4):
            s = 1024 * q
            nc.scalar.activation(
                out=bt[:, s:s + 1024],
                in_=bt[:, s:s + 1024],
                func=mybir.ActivationFunctionType.Tanh,
            )

        for ic in range(NCHUNK):
            xs = xpool.tile([P, L], F32)
            # loads ride the Sync HWDGE ring
            nc.sync.dma_start(out=xs, in_=x_in[ic * P:(ic + 1) * P, :])
            outs = opool.tile([P, L], F32)
            for q in range(4):  # 8 column-groups at a time (2 PSUM banks)
                tp = pt.tile([P, 1024], F32)
                for t in range(8):
                    j = 8 * q + t
                    nc.tensor.transpose(
                        tp[:, P * t:P * (t + 1)],
                        xs[:, P * j:P * (j + 1)],
                        ident,
                    )
                xt = xtpool.tile([P, 1024], F32)
                nc.vector.tensor_copy(out=xt, in_=tp)
                po = pm.tile([P, 1024], F32)
                for t in range(8):
                    g = 8 * q + t
                    nc.tensor.matmul(
                        po[:, P * t:P * (t + 1)],
                        lhsT=xt[:, P * t:P * (t + 1)],
                        rhs=bt[:, P * g:P * (g + 1)],
                        start=True,
                        stop=True,
                    )
                # PSUM->SBUF out copies: 1 of 4 on DVE, 3 of 4 on ACT
                if q == 0:
                    nc.vector.tensor_copy(out=outs[:, 1024 * q:1024 * (q + 1)], in_=po)
                else:
                    nc.scalar.copy(out=outs[:, 1024 * q:1024 * (q + 1)], in_=po)
            # stores ride the Scalar HWDGE ring (decoupled from loads)
            nc.scalar.dma_start(out=out_ext[ic * P:(ic + 1) * P, :], in_=outs)

    nc.compile()
    return nc


def _get_nc():
    if "nc" not in _NC_CACHE:
        _NC_CACHE["nc"] = build_nc()
    return _NC_CACHE["nc"]


def _mask_is_block_diagonal(mask: np.ndarray) -> bool:
    off = mask.copy()
    for i in range(NB):
        s = i * B
        off[s:s + B, s:s + B] = False
    return not off.any()


def _build_wt(W: np.ndarray, mask: np.ndarray) -> np.ndarray:
    """Pack the 64 diagonal blocks, transposed, into [128, 4096]:
    wt[r, 128*g + c] = (W*mask).T within block pair g, zeros off-block."""
    Wm = np.where(mask, W, np.float32(0.0)).astype(np.float32)
    wt_g = np.zeros((NG, P, P), dtype=np.float32)
    for g in range(NG):
        for h in range(2):
            b = 2 * g + h
            s = b * B
            blk = Wm[s:s + B, s:s + B]
            wt_g[g, h * B:(h + 1) * B, h * B:(h + 1) * B] = blk.T
    # [g, r, c] -> [r, g*128 + c]
    return np.ascontiguousarray(wt_g.transpose(1, 0, 2).reshape(P, L))


def run(x, W, mask, trace=False, trace_cores=None, tmpdir=None):
    x = np.ascontiguousarray(np.asarray(x, dtype=np.float32))
    W = np.asarray(W, dtype=np.float32)
    mask = np.asarray(mask).astype(bool)
    assert x.shape == (ROWS, L) and W.shape == (L, L) and mask.shape == (L, L)

    if not _mask_is_block_diagonal(mask):
        # Safety net (never expected): mask is not block-diagonal, so the
        # effective weight is dense. Compute densely on host.
        blocks = np.tanh(np.where(mask, W, np.float32(0.0)).astype(np.float32))
        return (x @ blocks.T).astype(np.float32), None

    wt = _build_wt(W, mask)
    ident = np.eye(P, dtype=np.float32)

    nc = _get_nc()
    in_maps = [
        {"x": x[c * R:(c + 1) * R, :], "wt": wt, "ident": ident}
        for c in range(NCORES)
    ]
    res = run_bass_kernel_spmd(
        nc,
        in_maps,
        list(range(NCORES)),
        trace=trace,
        trace_cores=trace_cores,
        tmpdir=tmpdir,
    )
    out = np.concatenate([res.results[c]["out"] for c in range(NCORES)], axis=0)
    return out, res


def kernel(x, W, mask):
    out, _ = run(x, W, mask, trace=False)
    return out


# revision 6
# speedup vs baseline: 1.0627x; 1.0627x over previous
"""Trainium2 Bass kernel for nn_BlockDiagonal.

Math: out = x @ tanh(W * mask).T, where mask is block-diagonal with 64
blocks of 64x64. tanh(0) = 0, so the effective weight is block-diagonal:
out[:, 64j:64j+64] = x[:, 64j:64j+64] @ tanh(Wb_j).T for each block j.

Strategy (8 NeuronCores, data parallel over rows):
- Each core owns 1024 rows of x / out.
- Host (numpy, data movement only): verify the mask is block-diagonal,
  gather the 64 diagonal 64x64 blocks of W, lay them out as 32 groups of
  two blocks packed into [128,128] block-diagonal tiles, TRANSPOSED, with
  zeros off-block -> wt [128, 4096]. tanh on device maps zeros to zeros.
- Device per core: tanh(wt) once; stream x in [128, 4096] chunks;
  PE-transpose each [128,128] x tile (fp32 transpose mode); fp32 matmuls
  lhsT = xT_tile [feat, rows], rhs = tanh-tile [feat_in, feat_out];
  copy PSUM->SBUF; DMA the [128, 4096] out chunk back.
"""

import os
import sys

import numpy as np

for _p in ("/opt/trn_rl_repo", "/root/.axon_site/_ro/trn_rl_repo"):
    if os.path.isdir(_p) and _p not in sys.path:
        sys.path.append(_p)

from contextlib import ExitStack

import concourse.bacc as bacc
from concourse import mybir
from concourse.bass_utils import run_bass_kernel_spmd
from concourse.tile import TileContext

ROWS = 8192
L = 4096
NB = 64           # number of diagonal blocks
B = 64            # block size
NCORES = 8
R = ROWS // NCORES  # rows per core = 1024
P = 128             # partition tile
NG = L // P         # 32 column groups (2 blocks each)
NCHUNK = R // P     # 8 row chunks per core

F32 = mybir.dt.float32

_NC_CACHE = {}


def build_nc():
    """Build the per-core Bass program (SPMD: same program, different data)."""
    nc = bacc.Bacc()
    x_in = nc.declare_dram_parameter("x", [R, L], F32, isOutput=False)
    wt_in = nc.declare_dram_parameter("wt", [P, L], F32, isOutput=False)
    id_in = nc.declare_dram_parameter("ident", [P, P], F32, isOutput=False)
    out_ext = nc.declare_dram_parameter("out", [R, L], F32, isOutput=True)

    with TileContext(nc) as tc, ExitStack() as ctx:
        singles = ctx.enter_context(tc.tile_pool(name="singles", bufs=1))
        xpool = ctx.enter_context(tc.tile_pool(name="xpool", bufs=4))
        xtpool = ctx.enter_context(tc.tile_pool(name="xtpool", bufs=6))
        opool = ctx.enter_context(tc.tile_pool(name="opool", bufs=3))
        pt = ctx.enter_context(tc.tile_pool(name="pt", bufs=2, space="PSUM"))
        pm = ctx.enter_context(tc.tile_pool(name="pm", bufs=2, space="PSUM"))

        ident = singles.tile([P, P], F32)
        nc.sync.dma_start(out=ident, in_=id_in[:])

        bt = singles.tile([P, L], F32)
        nc.sync.dma_start(out=bt, in_=wt_in[:])
        for q in range(4):
            s = 1024 * q
            nc.scalar.activation(
                out=bt[:, s:s + 1024],
                in_=bt[:, s:s + 1024],
                func=mybir.ActivationFunctionType.Tanh,
            )

        for ic in range(NCHUNK):
            last = ic == NCHUNK - 1
            xs = xpool.tile([P, L], F32)
            # loads ride the Sync HWDGE ring; half-chunk granularity
            nc.sync.dma_start(
                out=xs[:, 0:L // 2],
                in_=x_in[ic * P:(ic + 1) * P, 0:L // 2],
            )
            nc.sync.dma_start(
                out=xs[:, L // 2:L],
                in_=x_in[ic * P:(ic + 1) * P, L // 2:L],
            )
            outs = opool.tile([P, L], F32)
            for q in range(4):  # 8 column-groups at a time (2 PSUM banks)
                tp = pt.tile([P, 1024], F32)
                for t in range(8):
                    j = 8 * q + t
                    nc.tensor.transpose(
                        tp[:, P * t:P * (t + 1)],
                        xs[:, P * j:P * (j + 1)],
                        ident,
                    )
                xt = xtpool.tile([P, 1024], F32)
                nc.vector.tensor_copy(out=xt, in_=tp)
                po = pm.tile([P, 1024], F32)
                for t in range(8):
                    g = 8 * q + t
                    nc.tensor.matmul(
                        po[:, P * t:P * (t + 1)],
                        lhsT=xt[:, P * t:P * (t + 1)],
                        rhs=bt[:, P * g:P * (g + 1)],
                        start=True,
                        stop=True,
                    )
                # PSUM->SBUF out copies on ACT (DVE stays dedicated to
                # the xt copies that feed PE); last chunk splits to DVE
                # since no more xt copies are coming.
                if last and q % 2 == 1:
                    nc.vector.tensor_copy(out=outs[:, 1024 * q:1024 * (q + 1)], in_=po)
                else:
                    nc.scalar.copy(out=outs[:, 1024 * q:1024 * (q + 1)], in_=po)
                # stores ride the Scalar HWDGE ring, issued per half chunk
                if q % 2 == 1:
                    h = q // 2
                    nc.scalar.dma_start(
                        out=out_ext[ic * P:(ic + 1) * P, h * (L // 2):(h + 1) * (L // 2)],
                        in_=outs[:, h * (L // 2):(h + 1) * (L // 2)],
                    )

    nc.compile()
    return nc


def _get_nc():
    if "nc" not in _NC_CACHE:
        _NC_CACHE["nc"] = build_nc()
    return _NC_CACHE["nc"]


def _mask_is_block_diagonal(mask: np.ndarray) -> bool:
    off = mask.copy()
    for i in range(NB):
        s = i * B
        off[s:s + B, s:s + B] = False
    return not off.any()


def _build_wt(W: np.ndarray, mask: np.ndarray) -> np.ndarray:
    """Pack the 64 diagonal blocks, transposed, into [128, 4096]:
    wt[r, 128*g + c] = (W*mask).T within block pair g, zeros off-block."""
    Wm = np.where(mask, W, np.float32(0.0)).astype(np.float32)
    wt_g = np.zeros((NG, P, P), dtype=np.float32)
    for g in range(NG):
        for h in range(2):
            b = 2 * g + h
            s = b * B
            blk = Wm[s:s + B, s:s + B]
            wt_g[g, h * B:(h + 1) * B, h * B:(h + 1) * B] = blk.T
    # [g, r, c] -> [r, g*128 + c]
    return np.ascontiguousarray(wt_g.transpose(1, 0, 2).reshape(P, L))


def run(x, W, mask, trace=False, trace_cores=None, tmpdir=None):
    x = np.ascontiguousarray(np.asarray(x, dtype=np.float32))
    W = np.asarray(W, dtype=np.float32)
    mask = np.asarray(mask).astype(bool)
    assert x.shape == (ROWS, L) and W.shape == (L, L) and mask.shape == (L, L)

    if not _mask_is_block_diagonal(mask):
        # Safety net (never expected): mask is not block-diagonal, so the
        # effective weight is dense. Compute densely on host.
        blocks = np.tanh(np.where(mask, W, np.float32(0.0)).astype(np.float32))
        return (x @ blocks.T).astype(np.float32), None

    wt = _build_wt(W, mask)
    ident = np.eye(P, dtype=np.float32)

    nc = _get_nc()
    in_maps = [
        {"x": x[c * R:(c + 1) * R, :], "wt": wt, "ident": ident}
        for c in range(NCORES)
    ]
    res = run_bass_kernel_spmd(
        nc,
        in_maps,
        list(range(NCORES)),
        trace=trace,
        trace_cores=trace_cores,
        tmpdir=tmpdir,
    )
    out = np.concatenate([res.results[c]["out"] for c in range(NCORES)], axis=0)
    return out, res


def kernel(x, W, mask):
    out, _ = run(x, W, mask, trace=False)
    return out


# revision 8
# speedup vs baseline: 1.0806x; 1.0168x over previous
"""Trainium2 Bass kernel for nn_BlockDiagonal.

Math: out = x @ tanh(W * mask).T, where mask is block-diagonal with 64
blocks of 64x64. tanh(0) = 0, so the effective weight is block-diagonal:
out[:, 64j:64j+64] = x[:, 64j:64j+64] @ tanh(Wb_j).T for each block j.

Strategy (8 NeuronCores, data parallel over rows):
- Each core owns 1024 rows of x / out.
- Host (numpy, data movement only): verify the mask is block-diagonal,
  gather the 64 diagonal 64x64 blocks of W, lay them out as 32 groups of
  two blocks packed into [128,128] block-diagonal tiles, TRANSPOSED, with
  zeros off-block -> wt [128, 4096]. tanh on device maps zeros to zeros.
- Device per core: tanh(wt) once; stream x in [128, 4096] chunks;
  PE-transpose each [128,128] x tile (fp32 transpose mode); fp32 matmuls
  lhsT = xT_tile [feat, rows], rhs = tanh-tile [feat_in, feat_out];
  copy PSUM->SBUF; DMA the [128, 4096] out chunk back.
"""

import os
import sys

import numpy as np

for _p in ("/opt/trn_rl_repo", "/root/.axon_site/_ro/trn_rl_repo"):
    if os.path.isdir(_p) and _p not in sys.path:
        sys.path.append(_p)

from contextlib import ExitStack

import concourse.bacc as bacc
from concourse import mybir
from concourse.bass_utils import run_bass_kernel_spmd
from concourse.tile import TileContext

ROWS = 8192
L = 4096
NB = 64           # number of diagonal blocks
B = 64            # block size
NCORES = 8
R = ROWS // NCORES  # rows per core = 1024
P = 128             # partition tile
NG = L // P         # 32 column groups (2 blocks each)
NCHUNK = R // P     # 8 row chunks per core

F32 = mybir.dt.float32

_NC_CACHE = {}


def build_nc():
    """Build the per-core Bass program (SPMD: same program, different data)."""
    nc = bacc.Bacc()
    x_in = nc.declare_dram_parameter("x", [R, L], F32, isOutput=False)
    wt_in = nc.declare_dram_parameter("wt", [P, L], F32, isOutput=False)
    id_in = nc.declare_dram_parameter("ident", [P, P], F32, isOutput=False)
    out_ext = nc.declare_dram_parameter("out", [R, L], F32, isOutput=True)

    with TileContext(nc) as tc, ExitStack() as ctx:
        singles = ctx.enter_context(tc.tile_pool(name="singles", bufs=1))
        xpool = ctx.enter_context(tc.tile_pool(name="xpool", bufs=4))
        xtpool = ctx.enter_context(tc.tile_pool(name="xtpool", bufs=6))
        opool = ctx.enter_context(tc.tile_pool(name="opool", bufs=3))
        pt = ctx.enter_context(tc.tile_pool(name="pt", bufs=2, space="PSUM"))
        pm = ctx.enter_context(tc.tile_pool(name="pm", bufs=2, space="PSUM"))

        ident = singles.tile([P, P], F32)
        nc.sync.dma_start(out=ident, in_=id_in[:])

        # x chunk 0 loads FIRST so PE transposes start ASAP; weights follow
        # (tanh slices land just in time for the first matmuls).
        xs0 = xpool.tile([P, L], F32, tag="xs")
        nc.sync.dma_start(out=xs0[:, 0:L // 2], in_=x_in[0:P, 0:L // 2])
        nc.sync.dma_start(out=xs0[:, L // 2:L], in_=x_in[0:P, L // 2:L])

        bt = singles.tile([P, L], F32)
        # weights ride the Scalar ring so they don't queue behind x loads
        nc.scalar.dma_start(out=bt[:, 0:L // 2], in_=wt_in[:, 0:L // 2])
        nc.scalar.dma_start(out=bt[:, L // 2:L], in_=wt_in[:, L // 2:L])
        for q in range(8):
            s = 512 * q
            nc.scalar.activation(
                out=bt[:, s:s + 512],
                in_=bt[:, s:s + 512],
                func=mybir.ActivationFunctionType.Tanh,
            )

        for ic in range(NCHUNK):
            last = ic == NCHUNK - 1
            if ic == 0:
                xs = xs0
            else:
                xs = xpool.tile([P, L], F32, tag="xs")
                # loads ride the Sync HWDGE ring; half-chunk granularity
                nc.sync.dma_start(
                    out=xs[:, 0:L // 2],
                    in_=x_in[ic * P:(ic + 1) * P, 0:L // 2],
                )
                nc.sync.dma_start(
                    out=xs[:, L // 2:L],
                    in_=x_in[ic * P:(ic + 1) * P, L // 2:L],
                )
            outs = opool.tile([P, L], F32)
            for q in range(4):  # 8 column-groups at a time (2 PSUM banks)
                tp = pt.tile([P, 1024], F32)
                for t in range(8):
                    j = 8 * q + t
                    nc.tensor.transpose(
                        tp[:, P * t:P * (t + 1)],
                        xs[:, P * j:P * (j + 1)],
                        ident,
                    )
                xt = xtpool.tile([P, 1024], F32)
                nc.vector.tensor_copy(out=xt, in_=tp)
                po = pm.tile([P, 1024], F32)
                for t in range(8):
                    g = 8 * q + t
                    nc.tensor.matmul(
                        po[:, P * t:P * (t + 1)],
                        lhsT=xt[:, P * t:P * (t + 1)],
                        rhs=bt[:, P * g:P * (g + 1)],
                        start=True,
                        stop=True,
                    )
                # PSUM->SBUF out copies on ACT (DVE stays dedicated to
                # the xt copies that feed PE); last chunk splits to DVE
                # since no more xt copies are coming.
                if last and q % 2 == 1:
                    nc.vector.tensor_copy(out=outs[:, 1024 * q:1024 * (q + 1)], in_=po)
                else:
                    nc.scalar.copy(out=outs[:, 1024 * q:1024 * (q + 1)], in_=po)
                # stores ride the Scalar HWDGE ring; quarter-chunk stores on
                # the last chunk to shorten the drain tail
                if last:
                    nc.scalar.dma_start(
                        out=out_ext[ic * P:(ic + 1) * P, 1024 * q:1024 * (q + 1)],
                        in_=outs[:, 1024 * q:1024 * (q + 1)],
                    )
                elif q % 2 == 1:
                    h = q // 2
                    nc.scalar.dma_start(
                        out=out_ext[ic * P:(ic + 1) * P, h * (L // 2):(h + 1) * (L // 2)],
                        in_=outs[:, h * (L // 2):(h + 1) * (L // 2)],
                    )

    nc.compile()
    return nc


def _get_nc():
    if "nc" not in _NC_CACHE:
        _NC_CACHE["nc"] = build_nc()
    return _NC_CACHE["nc"]


def _mask_is_block_diagonal(mask: np.ndarray) -> bool:
    off = mask.copy()
    for i in range(NB):
        s = i * B
        off[s:s + B, s:s + B] = False
    return not off.any()


def _build_wt(W: np.ndarray, mask: np.ndarray) -> np.ndarray:
    """Pack the 64 diagonal blocks, transposed, into [128, 4096]:
    wt[r, 128*g + c] = (W*mask).T within block pair g, zeros off-block."""
    Wm = np.where(mask, W, np.float32(0.0)).astype(np.float32)
    wt_g = np.zeros((NG, P, P), dtype=np.float32)
    for g in range(NG):
        for h in range(2):
            b = 2 * g + h
            s = b * B
            blk = Wm[s:s + B, s:s + B]
            wt_g[g, h * B:(h + 1) * B, h * B:(h + 1) * B] = blk.T
    # [g, r, c] -> [r, g*128 + c]
    return np.ascontiguousarray(wt_g.transpose(1, 0, 2).reshape(P, L))


def run(x, W, mask, trace=False, trace_cores=None, tmpdir=None):
    x = np.ascontiguousarray(np.asarray(x, dtype=np.float32))
    W = np.asarray(W, dtype=np.float32)
    mask = np.asarray(mask).astype(bool)
    assert x.shape == (ROWS, L) and W.shape == (L, L) and mask.shape == (L, L)

    if not _mask_is_block_diagonal(mask):
        # Safety net (never expected): mask is not block-diagonal, so the
        # effective weight is dense. Compute densely on host.
        blocks = np.tanh(np.where(mask, W, np.float32(0.0)).astype(np.float32))
        return (x @ blocks.T).astype(np.float32), None

    wt = _build_wt(W, mask)
    ident = np.eye(P, dtype=np.float32)

    nc = _get_nc()
    in_maps = [
        {"x": x[c * R:(c + 1) * R, :], "wt": wt, "ident": ident}
        for c in range(NCORES)
    ]
    res = run_bass_kernel_spmd(
        nc,
        in_maps,
        list(range(NCORES)),
        trace=trace,
        trace_cores=trace_cores,
        tmpdir=tmpdir,
    )
    out = np.concatenate([res.results[c]["out"] for c in range(NCORES)], axis=0)
    return out, res


def kernel(x, W, mask):
    out, _ = run(x, W, mask, trace=False)
    return out
